# revision 20
# baseline (speedup 1.0000x reference)
"""BigBird attention (B=4, N=4096, D=1024, H=16, BS=64) on 8 TRN2 NeuronCores.

Sharding: batch (4-way) x head-group (2-way).  Core c handles batch c//2 and
heads [hg*8, hg*8+8) where hg = c%2 (d_model slice [hg*512, hg*512+512)).
Per core:
  pass A: QKV projections.  x.T tiles produced with PE transposes; q/k emitted
          transposed (qT/kT: [dl, n], head dim on partitions), v natural.
          The 1/sqrt(dh) score scale is folded into Wq/bq on the host.
  pass B: per-head BigBird attention (local sliding window + global-column
          softmax summed, then global-row full attention overwrite), writing
          ctx transposed into an SBUF-resident accumulator.
  pass C: row-parallel output projection -> partial outT [d_model, n].
Host combines: out[b] = outT(core 2b).T + outT(core 2b+1).T + bo.

The kernel is specialized (compiled) per global_indices value.
"""

import functools
import sys

import numpy as np

P = 128
BS = 64
NEG = -1e9


def _ensure_path():
    try:
        import concourse.bass  # noqa: F401
    except ImportError:
        sys.path.insert(0, "/opt/trn_rl_repo")


def _build(n, dmodel, dl, g0, g1):
    """Build the per-core Bass program.

    n: sequence length per core, dmodel: model dim (contraction for QKV,
    output dim for out-proj), dl: local (per-core) head dims = hpc*64.
    g0, g1: global block indices (compile-time constants).
    """
    _ensure_path()
    from contextlib import ExitStack

    import concourse.bass as bass  # noqa: F401
    import concourse.tile as tile
    from concourse import bacc, mybir
    from concourse.masks import make_identity

    f32 = mybir.dt.float32
    bf16 = mybir.dt.bfloat16
    AF = mybir.ActivationFunctionType
    AX = mybir.AxisListType.X

    nch = n // 512   # 512-column chunks of the sequence
    ndc = dmodel // P  # contraction chunks for QKV proj
    njt = dl // P      # row tiles of qT/kT
    hpc = dl // BS     # heads per core
    nt = n // P        # query tiles (2 blocks each)
    nkc = n // 512     # key chunks for global-row attention
    ndc2 = dl // P     # contraction chunks for out proj
    npad = (n + 2 * BS) // P  # padded v blocks

    nc = bacc.Bacc(None, target_bir_lowering=False, debug=False)

    x_d = nc.dram_tensor("x", [n, dmodel], bf16, kind="ExternalInput")
    wq_d = nc.dram_tensor("wqT", [dmodel, dl], bf16, kind="ExternalInput")
    wk_d = nc.dram_tensor("wkT", [dmodel, dl], bf16, kind="ExternalInput")
    wv_d = nc.dram_tensor("wvT", [dmodel, dl], bf16, kind="ExternalInput")
    wo_d = nc.dram_tensor("woT", [dl, dmodel], bf16, kind="ExternalInput")
    bq_d = nc.dram_tensor("bq", [dl], f32, kind="ExternalInput")
    bk_d = nc.dram_tensor("bk", [dl], f32, kind="ExternalInput")
    bv_d = nc.dram_tensor("bv", [dl], f32, kind="ExternalInput")
    out_d = nc.dram_tensor("outT", [dmodel, n], f32, kind="ExternalOutput")

    with tile.TileContext(nc) as tc, ExitStack() as top:
        dram = top.enter_context(tc.tile_pool(name="dram", bufs=1, space="DRAM"))
        qT_d = dram.tile([dl, n], bf16)
        kT_d = dram.tile([dl, n], bf16)
        v_d = dram.tile([n, dl], bf16)

        const = top.enter_context(tc.tile_pool(name="const", bufs=1))
        ident = const.tile([P, P], bf16)
        make_identity(nc, ident)
        identf = const.tile([P, P], f32)
        make_identity(nc, identf)
        ones1 = const.tile([1, BS], f32)
        nc.gpsimd.memset(ones1, 1.0)
        onesP = const.tile([1, P], f32)
        nc.gpsimd.memset(onesP, 1.0)
        # additive mask for the 2-block query tile vs 4-block key window
        mask = const.tile([P, 256], f32)
        nc.gpsimd.memset(mask, 0.0)
        nc.gpsimd.memset(mask[0:BS, 192:256], NEG)
        nc.gpsimd.memset(mask[BS:P, 0:BS], NEG)

        # ctx transposed accumulator: row (grp*128+p) = local head dim,
        # lives in SBUF through passes B and C.
        ctx_pool = top.enter_context(tc.tile_pool(name="ctx", bufs=1))
        ctxT = ctx_pool.tile([P, ndc2, n], bf16)

        # ---------------- pass A: projections ----------------
        with ExitStack() as ps:
            wpool = ps.enter_context(tc.tile_pool(name="wpool", bufs=1))
            wq_sb = wpool.tile([P, ndc, dl], bf16)
            wk_sb = wpool.tile([P, ndc, dl], bf16)
            wv_sb = wpool.tile([P, ndc, dl], bf16)
            nc.sync.dma_start(wq_sb, wq_d.rearrange("(a p) j -> p a j", p=P))
            nc.sync.dma_start(wk_sb, wk_d.rearrange("(a p) j -> p a j", p=P))
            nc.sync.dma_start(wv_sb, wv_d.rearrange("(a p) j -> p a j", p=P))
            bq_sb = wpool.tile([P, njt], f32)
            bk_sb = wpool.tile([P, njt], f32)
            nc.sync.dma_start(bq_sb, bq_d.rearrange("(a p) -> p a", p=P))
            nc.sync.dma_start(bk_sb, bk_d.rearrange("(a p) -> p a", p=P))
            bv_row = wpool.tile([1, dl], f32)
            nc.sync.dma_start(bv_row, bv_d.rearrange("(a j) -> a j", a=1))

            psA = ps.enter_context(tc.tile_pool(name="psA", bufs=4, space="PSUM"))

            # bv broadcast to [P, dl] via ones-matmul
            bvp = psA.tile([P, dl], f32, tag="ps_a")
            nc.tensor.matmul(bvp, onesP, bv_row, start=True, stop=True)
            bv_bc = wpool.tile([P, dl], f32)
            nc.vector.tensor_copy(bv_bc, bvp)

            xtpool = ps.enter_context(tc.tile_pool(name="xtpool", bufs=3))
            aout = ps.enter_context(tc.tile_pool(name="aout", bufs=4))

            for ch in range(nch):
                n0 = ch * 512
                xT = xtpool.tile([P, ndc, 512], bf16, tag="xT")
                for dc in range(ndc):
                    nc.sync.dma_start(
                        xT[:, dc, :],
                        x_d[n0 : n0 + 512, dc * P : (dc + 1) * P],
                        transpose=True,
                    )
                # qT / kT (transposed outputs, bias per-partition)
                for w_sb, b_sb, dst in ((wq_sb, bq_sb, qT_d), (wk_sb, bk_sb, kT_d)):
                    for jt in range(njt):
                        pp = psA.tile([P, 512], f32, tag="ps_a")
                        for dc in range(ndc):
                            nc.tensor.matmul(
                                pp,
                                w_sb[:, dc, jt * P : (jt + 1) * P],
                                xT[:, dc, :],
                                start=(dc == 0),
                                stop=(dc == ndc - 1),
                            )
                        ot = aout.tile([P, 512], bf16, tag="aout")
                        nc.scalar.activation(
                            ot, pp, AF.Identity, bias=b_sb[:, jt : jt + 1]
                        )
                        nc.sync.dma_start(
                            dst[jt * P : (jt + 1) * P, n0 : n0 + 512], ot
                        )
                # v (natural layout, bias broadcast along free dim)
                for ns in range(4):
                    pp = psA.tile([P, dl], f32, tag="ps_a")
                    for dc in range(ndc):
                        nc.tensor.matmul(
                            pp,
                            xT[:, dc, ns * P : (ns + 1) * P],
                            wv_sb[:, dc, :],
                            start=(dc == 0),
                            stop=(dc == ndc - 1),
                        )
                    ot = aout.tile([P, dl], bf16, tag="aout_v")
                    nc.vector.tensor_add(ot, pp, bv_bc)
                    nc.sync.dma_start(v_d[n0 + ns * P : n0 + (ns + 1) * P, :], ot)

        # ---------------- pass B: attention ----------------
        with ExitStack() as ps:
            hq = ps.enter_context(tc.tile_pool(name="hq", bufs=2))
            hk = ps.enter_context(tc.tile_pool(name="hk", bufs=2))
            hv = ps.enter_context(tc.tile_pool(name="hv", bufs=2))
            hvu = ps.enter_context(tc.tile_pool(name="hvu", bufs=2))
            gpool = ps.enter_context(tc.tile_pool(name="gpool", bufs=2))
            apool = ps.enter_context(tc.tile_pool(name="apool", bufs=4))
            atpool = ps.enter_context(tc.tile_pool(name="atpool", bufs=4))
            stat = ps.enter_context(tc.tile_pool(name="stat", bufs=4))
            psS = ps.enter_context(tc.tile_pool(name="psS", bufs=2, space="PSUM"))
            psT = ps.enter_context(tc.tile_pool(name="psT", bufs=3, space="PSUM"))
            psC = ps.enter_context(tc.tile_pool(name="psC", bufs=2, space="PSUM"))

            for h in range(hpc):
                r0 = h * BS
                p0, grp = (h % 2) * BS, h // 2
                qT_sb = hq.tile([BS, n], bf16, tag="hq")
                nc.sync.dma_start(qT_sb, qT_d[r0 : r0 + BS, :])
                kT_sb = hk.tile([BS, n + 2 * BS], bf16, tag="hk")
                nc.sync.dma_start(kT_sb[:, BS : BS + n], kT_d[r0 : r0 + BS, :])
                nc.sync.dma_start(kT_sb[:, 0:BS], kT_d[r0 : r0 + BS, n - BS : n])
                nc.sync.dma_start(kT_sb[:, BS + n :], kT_d[r0 : r0 + BS, 0:BS])
                vs = v_d[:, r0 : r0 + BS]
                v_sb = hv.tile([P, npad, BS], bf16, tag="hv")
                nc.sync.dma_start(v_sb[0:BS, 0, :], vs[n - BS : n, :])
                nc.sync.dma_start(v_sb[BS:P, 0, :], vs[0:BS, :])
                nc.sync.dma_start(
                    v_sb[:, 1 : npad - 1, :],
                    vs[BS : n - BS, :].rearrange("(a p) c -> p a c", p=P),
                )
                nc.sync.dma_start(v_sb[0:BS, npad - 1, :], vs[n - BS : n, :])
                nc.sync.dma_start(v_sb[BS:P, npad - 1, :], vs[0:BS, :])
                vu_sb = hvu.tile([P, n // P, BS], bf16, tag="hvu")
                nc.sync.dma_start(vu_sb, vs.rearrange("(a p) c -> p a c", p=P))
                # global key/value blocks (compact)
                kTg = gpool.tile([BS, 2 * BS], bf16, tag="kTg")
                vg = gpool.tile([P, BS], bf16, tag="vg")
                qg = gpool.tile([BS, P], bf16, tag="qg")
                for gi, gv in enumerate((g0, g1)):
                    nc.vector.tensor_copy(
                        kTg[:, gi * BS : (gi + 1) * BS],
                        kT_sb[:, BS + gv * BS : BS + (gv + 1) * BS],
                    )
                    nc.sync.dma_start(
                        vg[gi * BS : (gi + 1) * BS, :], vs[gv * BS : (gv + 1) * BS, :]
                    )
                    nc.vector.tensor_copy(
                        qg[:, gi * BS : (gi + 1) * BS],
                        qT_sb[:, gv * BS : (gv + 1) * BS],
                    )

                # ---- local window + global columns ----
                for t in range(nt):
                    sps = psS.tile([P, 512], f32, tag="sps")
                    qsl = qT_sb[:, t * P : (t + 1) * P]
                    nc.tensor.matmul(
                        sps[:, 0:256],
                        qsl,
                        kT_sb[:, t * P : t * P + 256],
                        start=True,
                        stop=True,
                    )
                    nc.tensor.matmul(sps[:, 256:384], qsl, kTg, start=True, stop=True)
                    nc.vector.tensor_add(sps[:, 0:256], sps[:, 0:256], mask)
                    negm = stat.tile([P, 2], f32, tag="negm")
                    nc.vector.reduce_max(
                        negm[:, 0:1], sps[:, 0:256], axis=AX, negate=True
                    )
                    nc.vector.reduce_max(
                        negm[:, 1:2], sps[:, 256:384], axis=AX, negate=True
                    )
                    s2 = stat.tile([P, 2], f32, tag="s2")
                    a_sb = apool.tile([P, 384], bf16, tag="a")
                    nc.scalar.activation(
                        a_sb[:, 0:256],
                        sps[:, 0:256],
                        AF.Exp,
                        bias=negm[:, 0:1],
                        accum_out=s2[:, 0:1],
                    )
                    nc.scalar.activation(
                        a_sb[:, 256:384],
                        sps[:, 256:384],
                        AF.Exp,
                        bias=negm[:, 1:2],
                        accum_out=s2[:, 1:2],
                    )
                    r2 = stat.tile([P, 2], f32, tag="r2")
                    nc.vector.reciprocal(r2, s2)
                    nc.vector.tensor_scalar_mul(
                        a_sb[:, 0:256], a_sb[:, 0:256], r2[:, 0:1]
                    )
                    nc.vector.tensor_scalar_mul(
                        a_sb[:, 256:384], a_sb[:, 256:384], r2[:, 1:2]
                    )
                    at_sb = atpool.tile([P, 3, P], bf16, tag="at")
                    nc.scalar.dma_start(at_sb, a_sb[:, 0:384], transpose=True)
                    cps = psC.tile([BS, P], f32, tag="cps")
                    nc.tensor.matmul(
                        cps, v_sb[:, t, :], at_sb[:, 0, :], start=True, stop=False
                    )
                    nc.tensor.matmul(
                        cps, v_sb[:, t + 1, :], at_sb[:, 1, :], start=False, stop=False
                    )
                    nc.tensor.matmul(cps, vg, at_sb[:, 2, :], start=False, stop=True)
                    nc.scalar.copy(ctxT[p0 : p0 + BS, grp, t * P : (t + 1) * P], cps)

                # ---- global rows: full attention, overwrite ----
                mr = stat.tile([P, nkc], f32, tag="mr")
                for kc in range(nkc):
                    sps = psS.tile([P, 512], f32, tag="sps")
                    nc.tensor.matmul(
                        sps,
                        qg,
                        kT_sb[:, BS + kc * 512 : BS + (kc + 1) * 512],
                        start=True,
                        stop=True,
                    )
                    nc.vector.reduce_max(mr[:, kc : kc + 1], sps, axis=AX)
                negmr = stat.tile([P, 1], f32, tag="negmr")
                nc.vector.reduce_max(negmr, mr, axis=AX, negate=True)
                sr = stat.tile([P, nkc], f32, tag="sr")
                crp = psC.tile([BS, P], f32, tag="crp", bufs=1)
                for kc in range(nkc):
                    sps = psS.tile([P, 512], f32, tag="sps")
                    nc.tensor.matmul(
                        sps,
                        qg,
                        kT_sb[:, BS + kc * 512 : BS + (kc + 1) * 512],
                        start=True,
                        stop=True,
                    )
                    ar = apool.tile([P, 512], bf16, tag="ar")
                    nc.scalar.activation(
                        ar, sps, AF.Exp, bias=negmr, accum_out=sr[:, kc : kc + 1]
                    )
                    atr = atpool.tile([P, 4, P], bf16, tag="atr")
                    nc.scalar.dma_start(atr, ar, transpose=True)
                    for cc in range(4):
                        nc.tensor.matmul(
                            crp,
                            vu_sb[:, kc * 4 + cc, :],
                            atr[:, cc, :],
                            start=(kc == 0 and cc == 0),
                            stop=(kc == nkc - 1 and cc == 3),
                        )
                srf = stat.tile([P, 1], f32, tag="srf")
                nc.vector.reduce_sum(srf, sr, axis=AX)
                rr = stat.tile([P, 1], f32, tag="rr")
                nc.vector.reciprocal(rr, srf)
                tpr = psT.tile([P, P], f32, tag="tp")
                nc.tensor.transpose(tpr[0:1, :], rr, identf)
                rrT = stat.tile([1, P], f32, tag="rrT")
                nc.vector.tensor_copy(rrT, tpr[0:1, :])
                rbc = psT.tile([P, P], f32, tag="tp")
                nc.tensor.matmul(rbc[0:BS, :], ones1, rrT, start=True, stop=True)
                rbc_sb = stat.tile([BS, P], f32, tag="rbc_sb")
                nc.scalar.copy(rbc_sb, rbc[0:BS, :])
                for gi, gv in enumerate((g0, g1)):
                    nc.vector.tensor_mul(
                        ctxT[p0 : p0 + BS, grp, gv * BS : (gv + 1) * BS],
                        crp[:, gi * BS : (gi + 1) * BS],
                        rbc_sb[:, gi * BS : (gi + 1) * BS],
                    )

        # ---------------- pass C: output projection ----------------
        with ExitStack() as ps:
            wop = ps.enter_context(tc.tile_pool(name="wop", bufs=1))
            wo_sb = wop.tile([P, ndc2, dmodel], bf16)
            nc.sync.dma_start(wo_sb, wo_d.rearrange("(a p) o -> p a o", p=P))
            copool = ps.enter_context(tc.tile_pool(name="co", bufs=4))
            psO = ps.enter_context(tc.tile_pool(name="psO", bufs=4, space="PSUM"))
            for ot in range(dmodel // P):
                for ncc in range(n // 512):
                    pp = psO.tile([P, 512], f32, tag="pso")
                    for dc in range(ndc2):
                        nc.tensor.matmul(
                            pp,
                            wo_sb[:, dc, ot * P : (ot + 1) * P],
                            ctxT[:, dc, ncc * 512 : (ncc + 1) * 512],
                            start=(dc == 0),
                            stop=(dc == ndc2 - 1),
                        )
                    ob = copool.tile([P, 512], f32, tag="ob")
                    nc.vector.tensor_copy(ob, pp)
                    nc.sync.dma_start(
                        out_d[ot * P : (ot + 1) * P, ncc * 512 : (ncc + 1) * 512], ob
                    )

    nc.finalize()
    return nc


@functools.lru_cache(maxsize=8)
def _get(n, dmodel, dl, g0, g1):
    return _build(n, dmodel, dl, g0, g1)


def _prepare(inputs):
    """Build (nc, in_maps, meta) for the SPMD run from full unsharded inputs."""
    x = np.asarray(inputs["x"], np.float32)
    Wq = np.asarray(inputs["Wq"], np.float32)
    Wk = np.asarray(inputs["Wk"], np.float32)
    Wv = np.asarray(inputs["Wv"], np.float32)
    Wo = np.asarray(inputs["Wo"], np.float32)
    bq = np.asarray(inputs["bq"], np.float32)
    bk = np.asarray(inputs["bk"], np.float32)
    bv = np.asarray(inputs["bv"], np.float32)
    bo = np.asarray(inputs["bo"], np.float32)
    gi = np.asarray(inputs["global_indices"]).astype(np.int64)
    g0, g1 = int(gi[0]), int(gi[1])

    b_, n_, d_ = x.shape
    dl = d_ // 2
    scale = 1.0 / np.sqrt(np.float32(64.0)).astype(np.float32)

    nc = _get(n_, d_, dl, g0, g1)

    import ml_dtypes

    bf = ml_dtypes.bfloat16
    in_maps = []
    for c in range(8):
        b, hg = divmod(c, 2)
        S = slice(hg * dl, (hg + 1) * dl)
        in_maps.append(
            {
                "x": np.ascontiguousarray(x[b]).astype(bf),
                "wqT": np.ascontiguousarray((Wq[S, :] * scale).T).astype(bf),
                "wkT": np.ascontiguousarray(Wk[S, :].T).astype(bf),
                "wvT": np.ascontiguousarray(Wv[S, :].T).astype(bf),
                "woT": np.ascontiguousarray(Wo[:, S].T).astype(bf),
                "bq": np.ascontiguousarray(bq[S] * scale),
                "bk": np.ascontiguousarray(bk[S]),
                "bv": np.ascontiguousarray(bv[S]),
            }
        )

    return nc, in_maps, (b_, n_, d_, bo)


def _combine(res, meta):
    b_, n_, d_, bo = meta
    out = np.empty((b_, n_, d_), np.float32)
    for b in range(b_):
        out[b] = res[2 * b]["outT"].T + res[2 * b + 1]["outT"].T + bo[None, :]
    return out


def kernel(**inputs):
    _ensure_path()
    from concourse.bass_utils import run_bass_kernel_spmd

    nc, in_maps, meta = _prepare(inputs)
    res = run_bass_kernel_spmd(nc, in_maps, list(range(8))).results
    return _combine(res, meta)


# revision 21
# speedup vs baseline: 1.1797x; 1.1797x over previous
"""BigBird attention (B=4, N=4096, D=1024, H=16, BS=64) on 8 TRN2 NeuronCores.

Sharding: batch (4-way) x head-group (2-way).  Core c handles batch c//2 and
heads [hg*8, hg*8+8) where hg = c%2 (d_model slice [hg*512, hg*512+512)).
Per core:
  pass A: QKV projections.  x.T tiles produced with PE transposes; q/k emitted
          transposed (qT/kT: [dl, n], head dim on partitions), v natural.
          The 1/sqrt(dh) score scale is folded into Wq/bq on the host.
  pass B: per-head BigBird attention (local sliding window + global-column
          softmax summed, then global-row full attention overwrite), writing
          ctx transposed into an SBUF-resident accumulator.
  pass C: row-parallel output projection -> partial outT [d_model, n].
Host combines: out[b] = outT(core 2b).T + outT(core 2b+1).T + bo.

The kernel is specialized (compiled) per global_indices value.
"""

import functools
import sys

import numpy as np

P = 128
BS = 64
NEG = -1e9


def _ensure_path():
    try:
        import concourse.bass  # noqa: F401
    except ImportError:
        sys.path.insert(0, "/opt/trn_rl_repo")


def _build(n, dmodel, dl, g0, g1):
    """Build the per-core Bass program.

    n: sequence length per core, dmodel: model dim (contraction for QKV,
    output dim for out-proj), dl: local (per-core) head dims = hpc*64.
    g0, g1: global block indices (compile-time constants).
    """
    _ensure_path()
    from contextlib import ExitStack

    import concourse.bass as bass  # noqa: F401
    import concourse.tile as tile
    from concourse import bacc, mybir
    from concourse.masks import make_identity

    f32 = mybir.dt.float32
    bf16 = mybir.dt.bfloat16
    AF = mybir.ActivationFunctionType
    AX = mybir.AxisListType.X

    nch = n // 512   # 512-column chunks of the sequence
    ndc = dmodel // P  # contraction chunks for QKV proj
    njt = dl // P      # row tiles of qT/kT
    hpc = dl // BS     # heads per core
    nt = n // P        # query tiles (2 blocks each)
    nkc = n // 512     # key chunks for global-row attention
    ndc2 = dl // P     # contraction chunks for out proj
    npad = (n + 2 * BS) // P  # padded v blocks

    nc = bacc.Bacc(None, target_bir_lowering=False, debug=False)

    x_d = nc.dram_tensor("x", [n, dmodel], bf16, kind="ExternalInput")
    wq_d = nc.dram_tensor("wqT", [dmodel, dl], bf16, kind="ExternalInput")
    wk_d = nc.dram_tensor("wkT", [dmodel, dl], bf16, kind="ExternalInput")
    wv_d = nc.dram_tensor("wvT", [dmodel, dl], bf16, kind="ExternalInput")
    wo_d = nc.dram_tensor("woT", [dl, dmodel], bf16, kind="ExternalInput")
    bq_d = nc.dram_tensor("bq", [dl], f32, kind="ExternalInput")
    bk_d = nc.dram_tensor("bk", [dl], f32, kind="ExternalInput")
    bv_d = nc.dram_tensor("bv", [dl], f32, kind="ExternalInput")
    out_d = nc.dram_tensor("outT", [dmodel, n], f32, kind="ExternalOutput")

    with tile.TileContext(nc) as tc, ExitStack() as top:
        dram = top.enter_context(tc.tile_pool(name="dram", bufs=1, space="DRAM"))
        qT_d = dram.tile([dl, n], bf16)
        kT_d = dram.tile([dl, n], bf16)
        v_d = dram.tile([n, dl], bf16)

        const = top.enter_context(tc.tile_pool(name="const", bufs=1))
        ident = const.tile([P, P], bf16)
        make_identity(nc, ident)
        identf = const.tile([P, P], f32)
        make_identity(nc, identf)
        ones1 = const.tile([1, BS], f32)
        nc.gpsimd.memset(ones1, 1.0)
        onesP = const.tile([1, P], f32)
        nc.gpsimd.memset(onesP, 1.0)
        # additive mask for the 2-block query tile vs 4-block key window
        mask = const.tile([P, 256], f32)
        nc.gpsimd.memset(mask, 0.0)
        nc.gpsimd.memset(mask[0:BS, 192:256], NEG)
        nc.gpsimd.memset(mask[BS:P, 0:BS], NEG)

        # ctx transposed accumulator: row (grp*128+p) = local head dim,
        # lives in SBUF through passes B and C.
        ctx_pool = top.enter_context(tc.tile_pool(name="ctx", bufs=1))
        ctxT = ctx_pool.tile([P, ndc2, n], bf16)

        # ---------------- pass A: projections ----------------
        with ExitStack() as ps:
            wpool = ps.enter_context(tc.tile_pool(name="wpool", bufs=1))
            wq_sb = wpool.tile([P, ndc, dl], bf16)
            wk_sb = wpool.tile([P, ndc, dl], bf16)
            wv_sb = wpool.tile([P, ndc, dl], bf16)
            nc.sync.dma_start(wq_sb, wq_d.rearrange("(a p) j -> p a j", p=P))
            nc.sync.dma_start(wk_sb, wk_d.rearrange("(a p) j -> p a j", p=P))
            nc.sync.dma_start(wv_sb, wv_d.rearrange("(a p) j -> p a j", p=P))
            bq_sb = wpool.tile([P, njt], f32)
            bk_sb = wpool.tile([P, njt], f32)
            nc.sync.dma_start(bq_sb, bq_d.rearrange("(a p) -> p a", p=P))
            nc.sync.dma_start(bk_sb, bk_d.rearrange("(a p) -> p a", p=P))
            bv_row = wpool.tile([1, dl], f32)
            nc.sync.dma_start(bv_row, bv_d.rearrange("(a j) -> a j", a=1))

            psA = ps.enter_context(tc.tile_pool(name="psA", bufs=4, space="PSUM"))

            # bv broadcast to [P, dl] via ones-matmul
            bvp = psA.tile([P, dl], f32, tag="ps_a")
            nc.tensor.matmul(bvp, onesP, bv_row, start=True, stop=True)
            bv_bc = wpool.tile([P, dl], f32)
            nc.vector.tensor_copy(bv_bc, bvp)

            xtpool = ps.enter_context(tc.tile_pool(name="xtpool", bufs=3))
            aout = ps.enter_context(tc.tile_pool(name="aout", bufs=4))

            for ch in range(nch):
                n0 = ch * 512
                xT = xtpool.tile([P, ndc, 512], bf16, tag="xT")
                for dc in range(ndc):
                    nc.sync.dma_start(
                        xT[:, dc, :],
                        x_d[n0 : n0 + 512, dc * P : (dc + 1) * P],
                        transpose=True,
                    )
                # qT / kT (transposed outputs, bias per-partition)
                for w_sb, b_sb, dst in ((wq_sb, bq_sb, qT_d), (wk_sb, bk_sb, kT_d)):
                    for jt in range(njt):
                        pp = psA.tile([P, 512], f32, tag="ps_a")
                        for dc in range(ndc):
                            nc.tensor.matmul(
                                pp,
                                w_sb[:, dc, jt * P : (jt + 1) * P],
                                xT[:, dc, :],
                                start=(dc == 0),
                                stop=(dc == ndc - 1),
                            )
                        ot = aout.tile([P, 512], bf16, tag="aout")
                        nc.scalar.activation(
                            ot, pp, AF.Identity, bias=b_sb[:, jt : jt + 1]
                        )
                        nc.sync.dma_start(
                            dst[jt * P : (jt + 1) * P, n0 : n0 + 512], ot
                        )
                # v (natural layout, bias broadcast along free dim)
                for ns in range(4):
                    pp = psA.tile([P, dl], f32, tag="ps_a")
                    for dc in range(ndc):
                        nc.tensor.matmul(
                            pp,
                            xT[:, dc, ns * P : (ns + 1) * P],
                            wv_sb[:, dc, :],
                            start=(dc == 0),
                            stop=(dc == ndc - 1),
                        )
                    ot = aout.tile([P, dl], bf16, tag="aout_v")
                    nc.vector.tensor_add(ot, pp, bv_bc)
                    nc.sync.dma_start(v_d[n0 + ns * P : n0 + (ns + 1) * P, :], ot)

        # ---------------- pass B: attention ----------------
        with ExitStack() as ps:
            hq = ps.enter_context(tc.tile_pool(name="hq", bufs=2))
            hk = ps.enter_context(tc.tile_pool(name="hk", bufs=2))
            hv = ps.enter_context(tc.tile_pool(name="hv", bufs=2))
            hvu = ps.enter_context(tc.tile_pool(name="hvu", bufs=2))
            gpool = ps.enter_context(tc.tile_pool(name="gpool", bufs=2))
            apool = ps.enter_context(tc.tile_pool(name="apool", bufs=6))
            atpool = ps.enter_context(tc.tile_pool(name="atpool", bufs=6))
            stat = ps.enter_context(tc.tile_pool(name="stat", bufs=6))
            psS = ps.enter_context(tc.tile_pool(name="psS", bufs=4, space="PSUM"))
            psT = ps.enter_context(tc.tile_pool(name="psT", bufs=1, space="PSUM"))
            psC = ps.enter_context(tc.tile_pool(name="psC", bufs=2, space="PSUM"))

            for h in range(hpc):
                r0 = h * BS
                p0, grp = (h % 2) * BS, h // 2
                qT_sb = hq.tile([BS, n], bf16, tag="hq")
                nc.sync.dma_start(qT_sb, qT_d[r0 : r0 + BS, :])
                kT_sb = hk.tile([BS, n + 2 * BS], bf16, tag="hk")
                nc.sync.dma_start(kT_sb[:, BS : BS + n], kT_d[r0 : r0 + BS, :])
                nc.sync.dma_start(kT_sb[:, 0:BS], kT_d[r0 : r0 + BS, n - BS : n])
                nc.sync.dma_start(kT_sb[:, BS + n :], kT_d[r0 : r0 + BS, 0:BS])
                vs = v_d[:, r0 : r0 + BS]
                v_sb = hv.tile([P, npad, BS], bf16, tag="hv")
                nc.sync.dma_start(v_sb[0:BS, 0, :], vs[n - BS : n, :])
                nc.sync.dma_start(v_sb[BS:P, 0, :], vs[0:BS, :])
                nc.sync.dma_start(
                    v_sb[:, 1 : npad - 1, :],
                    vs[BS : n - BS, :].rearrange("(a p) c -> p a c", p=P),
                )
                nc.sync.dma_start(v_sb[0:BS, npad - 1, :], vs[n - BS : n, :])
                nc.sync.dma_start(v_sb[BS:P, npad - 1, :], vs[0:BS, :])
                vu_sb = hvu.tile([P, n // P, BS], bf16, tag="hvu")
                nc.sync.dma_start(vu_sb, vs.rearrange("(a p) c -> p a c", p=P))
                # global key/value blocks (compact)
                kTg = gpool.tile([BS, 2 * BS], bf16, tag="kTg")
                vg = gpool.tile([P, BS], bf16, tag="vg")
                qg = gpool.tile([BS, P], bf16, tag="qg")
                for gi, gv in enumerate((g0, g1)):
                    nc.vector.tensor_copy(
                        kTg[:, gi * BS : (gi + 1) * BS],
                        kT_sb[:, BS + gv * BS : BS + (gv + 1) * BS],
                    )
                    nc.sync.dma_start(
                        vg[gi * BS : (gi + 1) * BS, :], vs[gv * BS : (gv + 1) * BS, :]
                    )
                    nc.vector.tensor_copy(
                        qg[:, gi * BS : (gi + 1) * BS],
                        qT_sb[:, gv * BS : (gv + 1) * BS],
                    )

                # ---- local window + global columns ----
                for t in range(nt):
                    sps = psS.tile([P, 512], f32, tag="sps")
                    qsl = qT_sb[:, t * P : (t + 1) * P]
                    nc.tensor.matmul(
                        sps[:, 0:256],
                        qsl,
                        kT_sb[:, t * P : t * P + 256],
                        start=True,
                        stop=True,
                    )
                    nc.tensor.matmul(sps[:, 256:384], qsl, kTg, start=True, stop=True)
                    nc.vector.tensor_add(sps[:, 0:256], sps[:, 0:256], mask)
                    negm = stat.tile([P, 2], f32, tag="negm")
                    nc.vector.reduce_max(
                        negm[:, 0:1], sps[:, 0:256], axis=AX, negate=True
                    )
                    nc.vector.reduce_max(
                        negm[:, 1:2], sps[:, 256:384], axis=AX, negate=True
                    )
                    s2 = stat.tile([P, 2], f32, tag="s2")
                    a_sb = apool.tile([P, 384], bf16, tag="a")
                    nc.scalar.activation(
                        a_sb[:, 0:256],
                        sps[:, 0:256],
                        AF.Exp,
                        bias=negm[:, 0:1],
                        accum_out=s2[:, 0:1],
                    )
                    nc.scalar.activation(
                        a_sb[:, 256:384],
                        sps[:, 256:384],
                        AF.Exp,
                        bias=negm[:, 1:2],
                        accum_out=s2[:, 1:2],
                    )
                    r2 = stat.tile([P, 2], f32, tag="r2")
                    nc.vector.reciprocal(r2, s2)
                    nc.vector.tensor_scalar_mul(
                        a_sb[:, 0:256], a_sb[:, 0:256], r2[:, 0:1]
                    )
                    nc.vector.tensor_scalar_mul(
                        a_sb[:, 256:384], a_sb[:, 256:384], r2[:, 1:2]
                    )
                    at_sb = atpool.tile([P, 3, P], bf16, tag="at")
                    nc.sync.dma_start(at_sb, a_sb[:, 0:384], transpose=True)
                    cps = psC.tile([BS, P], f32, tag="cps")
                    nc.tensor.matmul(
                        cps, v_sb[:, t, :], at_sb[:, 0, :], start=True, stop=False
                    )
                    nc.tensor.matmul(
                        cps, v_sb[:, t + 1, :], at_sb[:, 1, :], start=False, stop=False
                    )
                    nc.tensor.matmul(cps, vg, at_sb[:, 2, :], start=False, stop=True)
                    nc.vector.tensor_copy(
                        ctxT[p0 : p0 + BS, grp, t * P : (t + 1) * P], cps
                    )

                # ---- global rows: full attention, overwrite ----
                mr = stat.tile([P, nkc], f32, tag="mr")
                for kc in range(nkc):
                    sps = psS.tile([P, 512], f32, tag="sps")
                    nc.tensor.matmul(
                        sps,
                        qg,
                        kT_sb[:, BS + kc * 512 : BS + (kc + 1) * 512],
                        start=True,
                        stop=True,
                    )
                    nc.vector.reduce_max(mr[:, kc : kc + 1], sps, axis=AX)
                negmr = stat.tile([P, 1], f32, tag="negmr")
                nc.vector.reduce_max(negmr, mr, axis=AX, negate=True)
                sr = stat.tile([P, nkc], f32, tag="sr")
                crp = psC.tile([BS, P], f32, tag="crp", bufs=1)
                for kc in range(nkc):
                    sps = psS.tile([P, 512], f32, tag="sps")
                    nc.tensor.matmul(
                        sps,
                        qg,
                        kT_sb[:, BS + kc * 512 : BS + (kc + 1) * 512],
                        start=True,
                        stop=True,
                    )
                    ar = apool.tile([P, 512], bf16, tag="ar")
                    nc.scalar.activation(
                        ar, sps, AF.Exp, bias=negmr, accum_out=sr[:, kc : kc + 1]
                    )
                    atr = atpool.tile([P, 4, P], bf16, tag="atr")
                    nc.sync.dma_start(atr, ar, transpose=True)
                    for cc in range(4):
                        nc.tensor.matmul(
                            crp,
                            vu_sb[:, kc * 4 + cc, :],
                            atr[:, cc, :],
                            start=(kc == 0 and cc == 0),
                            stop=(kc == nkc - 1 and cc == 3),
                        )
                srf = stat.tile([P, 1], f32, tag="srf")
                nc.vector.reduce_sum(srf, sr, axis=AX)
                rr = stat.tile([P, 1], f32, tag="rr")
                nc.vector.reciprocal(rr, srf)
                tpr = psT.tile([P, P], f32, tag="tp")
                nc.tensor.transpose(tpr[0:1, :], rr, identf)
                rrT = stat.tile([1, P], f32, tag="rrT")
                nc.vector.tensor_copy(rrT, tpr[0:1, :])
                rbc = psT.tile([P, P], f32, tag="tp")
                nc.tensor.matmul(rbc[0:BS, :], ones1, rrT, start=True, stop=True)
                rbc_sb = stat.tile([BS, P], f32, tag="rbc_sb")
                nc.scalar.copy(rbc_sb, rbc[0:BS, :])
                for gi, gv in enumerate((g0, g1)):
                    nc.vector.tensor_mul(
                        ctxT[p0 : p0 + BS, grp, gv * BS : (gv + 1) * BS],
                        crp[:, gi * BS : (gi + 1) * BS],
                        rbc_sb[:, gi * BS : (gi + 1) * BS],
                    )

        # ---------------- pass C: output projection ----------------
        with ExitStack() as ps:
            wop = ps.enter_context(tc.tile_pool(name="wop", bufs=1))
            wo_sb = wop.tile([P, ndc2, dmodel], bf16)
            nc.sync.dma_start(wo_sb, wo_d.rearrange("(a p) o -> p a o", p=P))
            copool = ps.enter_context(tc.tile_pool(name="co", bufs=4))
            psO = ps.enter_context(tc.tile_pool(name="psO", bufs=4, space="PSUM"))
            for ot in range(dmodel // P):
                for ncc in range(n // 512):
                    pp = psO.tile([P, 512], f32, tag="pso")
                    for dc in range(ndc2):
                        nc.tensor.matmul(
                            pp,
                            wo_sb[:, dc, ot * P : (ot + 1) * P],
                            ctxT[:, dc, ncc * 512 : (ncc + 1) * 512],
                            start=(dc == 0),
                            stop=(dc == ndc2 - 1),
                        )
                    ob = copool.tile([P, 512], f32, tag="ob")
                    nc.vector.tensor_copy(ob, pp)
                    nc.sync.dma_start(
                        out_d[ot * P : (ot + 1) * P, ncc * 512 : (ncc + 1) * 512], ob
                    )

    nc.finalize()
    return nc


@functools.lru_cache(maxsize=8)
def _get(n, dmodel, dl, g0, g1):
    return _build(n, dmodel, dl, g0, g1)


def _prepare(inputs):
    """Build (nc, in_maps, meta) for the SPMD run from full unsharded inputs."""
    x = np.asarray(inputs["x"], np.float32)
    Wq = np.asarray(inputs["Wq"], np.float32)
    Wk = np.asarray(inputs["Wk"], np.float32)
    Wv = np.asarray(inputs["Wv"], np.float32)
    Wo = np.asarray(inputs["Wo"], np.float32)
    bq = np.asarray(inputs["bq"], np.float32)
    bk = np.asarray(inputs["bk"], np.float32)
    bv = np.asarray(inputs["bv"], np.float32)
    bo = np.asarray(inputs["bo"], np.float32)
    gi = np.asarray(inputs["global_indices"]).astype(np.int64)
    g0, g1 = int(gi[0]), int(gi[1])

    b_, n_, d_ = x.shape
    dl = d_ // 2
    scale = 1.0 / np.sqrt(np.float32(64.0)).astype(np.float32)

    nc = _get(n_, d_, dl, g0, g1)

    import ml_dtypes

    bf = ml_dtypes.bfloat16
    in_maps = []
    for c in range(8):
        b, hg = divmod(c, 2)
        S = slice(hg * dl, (hg + 1) * dl)
        in_maps.append(
            {
                "x": np.ascontiguousarray(x[b]).astype(bf),
                "wqT": np.ascontiguousarray((Wq[S, :] * scale).T).astype(bf),
                "wkT": np.ascontiguousarray(Wk[S, :].T).astype(bf),
                "wvT": np.ascontiguousarray(Wv[S, :].T).astype(bf),
                "woT": np.ascontiguousarray(Wo[:, S].T).astype(bf),
                "bq": np.ascontiguousarray(bq[S] * scale),
                "bk": np.ascontiguousarray(bk[S]),
                "bv": np.ascontiguousarray(bv[S]),
            }
        )

    return nc, in_maps, (b_, n_, d_, bo)


def _combine(res, meta):
    b_, n_, d_, bo = meta
    out = np.empty((b_, n_, d_), np.float32)
    for b in range(b_):
        out[b] = res[2 * b]["outT"].T + res[2 * b + 1]["outT"].T + bo[None, :]
    return out


def kernel(**inputs):
    _ensure_path()
    from concourse.bass_utils import run_bass_kernel_spmd

    nc, in_maps, meta = _prepare(inputs)
    res = run_bass_kernel_spmd(nc, in_maps, list(range(8))).results
    return _combine(res, meta)
